# revision 34
# baseline (speedup 1.0000x reference)
"""AdaptiveFlowRouter (soft-MoE routing) on 8 Trainium2 NeuronCores.

Reference computation (per token t of B*S=8192, D=1024, P=8 experts):
    w      = softmax(x @ sel_w.T + sel_b)            # (t, P)
    inten  = sigmoid(x @ int_w.T + int_b)            # (t, 1)
    out    = inten * sum_p w_p * (x @ A_p.T)         # (t, D)
    pattern_entropy     = mean_t(-sum_p w log(w+1e-8))
    flow_intensity_mean = mean(inten)

Sharding: data-parallel over tokens (1024 tokens/core), expert weights
replicated -> no cross-core communication. Host pre-transposes x and
flow_patterns into the [K, *] layouts the TensorEngine wants and
pre-casts the matmul operands to bf16 (fp32 PSUM accumulation).
Each core returns its (1024, 1024) output slice plus a tiny stats tile
(per-token-entropy partials and intensity values); the host finishes
the two scalar means exactly.
"""

from contextlib import ExitStack

import ml_dtypes
import numpy as np

import concourse.bacc as bacc
import concourse.mybir as mybir
import concourse.tile as tile
from concourse.bass_utils import run_bass_kernel_spmd

B, S, D, P = 4, 2048, 1024, 8
N_CORES = 8
TOK = B * S            # 8192 tokens total
T = TOK // N_CORES     # 1024 tokens per core
NT = T // 128          # 8 token tiles (M blocks)
NJ = D // 128          # 8 contraction tiles (K blocks)
IB = 512               # PSUM free-dim block (one bank of f32)
NI = D // IB           # 2 output blocks

BF16 = mybir.dt.bfloat16
F32 = mybir.dt.float32
AF = mybir.ActivationFunctionType
ALU = mybir.AluOpType
AX = mybir.AxisListType

_CACHE = {}

def _build_nc():
    nc = bacc.Bacc()
    xt_d = nc.declare_dram_parameter("xt", [128, NJ * T], BF16, isOutput=False)
    at_d = nc.declare_dram_parameter("at", [P, 128, NJ * D], BF16, isOutput=False)
    si_d = nc.declare_dram_parameter("selint", [128, NJ * 9], BF16, isOutput=False)
    bb_d = nc.declare_dram_parameter("biasb", [128, 9], F32, isOutput=False)
    out_d = nc.declare_dram_parameter("out", [T, D], BF16, isOutput=True)
    st_d = nc.declare_dram_parameter("stats", [128, 16], F32, isOutput=True)

    ctx = ExitStack()
    with ctx:
        # --- Pre-TileContext DMA: expert-0's weights stream during the ~7us
        # framework preamble, issued from the ACT queue (only 4 issues, so
        # ACT still joins the init barrier on time; more pre-issues would
        # delay the barrier and cancel the gain). The consumer carries a
        # manually attached sem wait (added post-scheduling).
        at0_raw = ctx.enter_context(nc.sbuf_tensor("at0_sb", [128, NJ * D], BF16))
        sem_pre = nc.alloc_semaphore("sem_pre")
        HD = D // 2
        AQ = NJ * D // 4
        for q in range(4):
            nc.scalar.dma_start(
                at0_raw[:, q * AQ : (q + 1) * AQ], at_d[0][:, q * AQ : (q + 1) * AQ]
            ).then_inc(sem_pre, 16)

        first_mm_cell = []
        tc = ctx.enter_context(tile.TileContext(nc))
        const = ctx.enter_context(tc.tile_pool(name="const", bufs=1))
        work = ctx.enter_context(tc.tile_pool(name="work", bufs=2))
        psel = ctx.enter_context(tc.tile_pool(name="psel", bufs=2, space="PSUM"))
        pexp = ctx.enter_context(tc.tile_pool(name="pexp", bufs=6, space="PSUM"))

        si = const.tile([128, NJ * 9], BF16, tag="si", name="si")
        nc.sync.dma_start(si[:], si_d[:])
        xt_sb = [
            const.tile([128, D], BF16, tag=f"xt{t}", name=f"xt{t}")
            for t in range(NT)
        ]

        def lhs_ap(tt, jt):
            return xt_sb[tt][:, jt * 128 : (jt + 1) * 128]

        at_sb = [at0_raw]
        for p in range(1, P):
            a = const.tile([128, NJ * D], BF16, tag=f"at{p}", name=f"at{p}")
            at_sb.append(a)

        QD = D // 4
        for t in range(2):
            for h in range(4):
                nc.sync.dma_start(
                    xt_sb[t][:, h * QD : (h + 1) * QD],
                    xt_d[:, t * D + h * QD : t * D + (h + 1) * QD],
                )
        bb = const.tile([128, 9], F32, tag="bb", name="bb")
        nc.sync.dma_start(bb[:], bb_d[:])
        for t in range(2, NT):
            nc.sync.dma_start(xt_sb[t][:], xt_d[:, t * D : (t + 1) * D])
        AH = NJ * D // 2
        for p in range(1, P):
            for h in range(2):
                nc.sync.dma_start(
                    at_sb[p][:, h * AH : (h + 1) * AH],
                    at_d[p][:, h * AH : (h + 1) * AH],
                )
        # stats: cols 0..7 = sum_p w*log(w+1e-8) per token, cols 8..15 = intensity
        stats = const.tile([128, 16], F32, tag="stats", name="stats")
        eps = const.tile([128, 1], F32, tag="eps", name="eps")
        nc.vector.memset(eps[:], 1e-8)
        # per-token mixing weights, pre-scaled by intensity: wsc[t, p] = w_p * inten
        wsc = const.tile([128, NT * P], F32, tag="wsc", name="wsc")
        mix_all = const.tile([128, NT * D], F32, tag="mix", name="mix")
        mix = [mix_all[:, t * D : (t + 1) * D] for t in range(NT)]

        def sel_tt(tt):
            ps = psel.tile([128, 9], F32, tag="ps", name="ps")
            for jt in range(NJ):
                mm = nc.tensor.matmul(
                    ps[:],
                    lhs_ap(tt, jt),
                    si[:, jt * 9 : (jt + 1) * 9],
                    start=(jt == 0),
                    stop=(jt == NJ - 1),
                )


            sc = work.tile([128, 48], F32, tag="softsc", name="softsc")
            logits = sc[:, 0:9]
            negm = sc[:, 9:10]
            e = sc[:, 10:18]
            esum = sc[:, 18:19]
            rcp = sc[:, 19:20]
            coef = sc[:, 20:21]
            wtrue = sc[:, 21:29]
            lw = sc[:, 29:37]
            went = sc[:, 37:45]
            nc.vector.tensor_add(logits, ps[:], bb[:])
            nc.vector.tensor_reduce(
                negm, logits[:, 0:8], axis=AX.X, op=ALU.max, negate=True
            )
            nc.scalar.activation(
                e, logits[:, 0:8], AF.Exp, bias=negm, accum_out=esum
            )
            nc.vector.reciprocal(rcp, esum)
            nc.scalar.activation(
                stats[:, 8 + tt : 9 + tt], logits[:, 8:9], AF.Sigmoid
            )
            nc.vector.tensor_mul(coef, rcp, stats[:, 8 + tt : 9 + tt])
            nc.vector.tensor_scalar_mul(
                wsc[:, tt * P : (tt + 1) * P], e, coef
            )
            nc.vector.tensor_scalar_mul(wtrue, e, rcp)
            nc.scalar.activation(lw, wtrue, AF.Ln, bias=eps[:])
            nc.vector.scalar_tensor_tensor(
                went,
                wtrue,
                1.0,
                lw,
                op0=ALU.bypass,
                op1=ALU.mult,
                accum_out=stats[:, tt : tt + 1],
            )

        def expert_tt(p, tt):
            pt = [pexp.tile([128, IB], F32, tag="pe", name="pe") for _ in range(NI)]
            for jt in range(NJ):
                for ib in range(NI):
                    mm = nc.tensor.matmul(
                        pt[ib][:],
                        lhs_ap(tt, jt),
                        at_sb[p][:, jt * D + ib * IB : jt * D + (ib + 1) * IB],
                        start=(jt == 0),
                        stop=(jt == NJ - 1),
                    )
                    if p == 0 and tt == 0 and jt == 0 and ib == 0:
                        first_mm_cell.append(mm)
            wcol = wsc[:, tt * P + p : tt * P + p + 1]
            if p == P - 1:
                # last expert: fused mul-add emits the final value in bf16
                # directly (DVE converts on write), halving store traffic
                ob = work.tile([128, D], BF16, tag="ob", name="ob", bufs=2)
                for ib in range(NI):
                    nc.vector.scalar_tensor_tensor(
                        ob[:, ib * IB : (ib + 1) * IB],
                        pt[ib][:],
                        wcol,
                        mix[tt][:, ib * IB : (ib + 1) * IB],
                        op0=ALU.mult,
                        op1=ALU.add,
                    )
                    nc.sync.dma_start(
                        out_d[tt * 128 : (tt + 1) * 128, ib * IB : (ib + 1) * IB],
                        ob[:, ib * IB : (ib + 1) * IB],
                    )
            else:
                for ib in range(NI):
                    dst = mix[tt][:, ib * IB : (ib + 1) * IB]
                    if p == 0:
                        nc.vector.tensor_scalar_mul(dst, pt[ib][:], wcol)
                    else:
                        nc.vector.scalar_tensor_tensor(
                            dst, pt[ib][:], wcol, dst, op0=ALU.mult, op1=ALU.add
                        )

        # PE p-state warmup: the first ~14us are input-transfer-bound, and an
        # idle PE drops to 0.65-1.2GHz. Dummy matmuls on a zeroed tile keep
        # the clock ramping so real matmuls start at full speed.
        warm = const.tile([128, 128], BF16, tag="warm", name="warm")
        nc.vector.memset(warm[:], 0.0)

        def warmup(n):
            wp = pexp.tile([128, IB], F32, tag="pe", name="pe")
            for k in range(n):
                nc.tensor.matmul(
                    wp[:, 0:128], warm[:], warm[:], start=True, stop=True
                )

        # Interleave selector and expert-0 per xt chunk so the PE rides the
        # incoming x/weight stream, then run experts 1..7 expert-major.
        warmup(24)
        sel_tt(0)
        sel_tt(1)
        expert_tt(0, 0)
        expert_tt(0, 1)
        for c in range(1, 4):
            sel_tt(2 * c)
            sel_tt(2 * c + 1)
            expert_tt(0, 2 * c)
            expert_tt(0, 2 * c + 1)
        nc.sync.dma_start(st_d[:], stats[:])
        for p in range(1, P):
            for tt in range(NT):
                expert_tt(p, tt)
        ctx.close()
        # Attach the external gate after Tile scheduling (the scheduler's
        # deadlock model cannot see pre-region DMA increments).
        first_mm_cell[0]._wait_ge(sem_pre, 64)
        nc.clear_and_free_semaphores([sem_pre])
    nc.finalize()
    return nc


def _prep_in_maps(x, flow_patterns, sel_w, sel_b, int_w, int_b):
    bf = ml_dtypes.bfloat16
    f32 = np.float32
    x = np.asarray(x, f32)
    flow_patterns = np.asarray(flow_patterns, f32)
    sel_w = np.asarray(sel_w, f32)
    sel_b = np.asarray(sel_b, f32)
    int_w = np.asarray(int_w, f32)
    int_b = np.asarray(int_b, f32)

    # A_p^T in [j, i] layout as the per-expert SBUF image:
    # at_h[p, part, jt*D + i] = A_p[i, jt*128 + part]
    at_h = np.ascontiguousarray(
        flow_patterns.transpose(0, 2, 1)
        .reshape(P, NJ, 128, D)
        .transpose(0, 2, 1, 3)
        .reshape(P, 128, NJ * D)
    ).astype(bf)
    si_h = np.ascontiguousarray(
        np.concatenate([sel_w, int_w], axis=0)
        .T.reshape(NJ, 128, 9)
        .transpose(1, 0, 2)
        .reshape(128, NJ * 9)
    ).astype(bf)
    bb_h = np.ascontiguousarray(
        np.broadcast_to(np.concatenate([sel_b, int_b], axis=0)[None, :], (128, 9))
    ).astype(f32)

    xf = x.reshape(TOK, D)
    in_maps = []
    for c in range(N_CORES):
        xt_h = np.ascontiguousarray(
            xf[c * T : (c + 1) * T]
            .T.reshape(NJ, 128, NT, 128)
            .transpose(1, 2, 0, 3)
            .reshape(128, NJ * T)
        ).astype(bf)
        in_maps.append({"xt": xt_h, "at": at_h, "selint": si_h, "biasb": bb_h})
    return in_maps


def _gather(results):
    out = (
        np.concatenate([r["out"] for r in results], axis=0)
        .astype(np.float32)
        .reshape(B, S, D)
    )
    ent_sum = sum(float(r["stats"][:, 0:8].sum(dtype=np.float64)) for r in results)
    int_sum = sum(float(r["stats"][:, 8:16].sum(dtype=np.float64)) for r in results)
    pattern_entropy = np.float32(-ent_sum / TOK)
    flow_intensity_mean = np.float32(int_sum / TOK)
    return out, pattern_entropy, flow_intensity_mean


def kernel(x, flow_patterns, sel_w, sel_b, int_w, int_b):
    if "nc" not in _CACHE:
        _CACHE["nc"] = _build_nc()
    nc = _CACHE["nc"]
    in_maps = _prep_in_maps(x, flow_patterns, sel_w, sel_b, int_w, int_b)
    res = run_bass_kernel_spmd(nc, in_maps, core_ids=list(range(N_CORES)))
    _CACHE["last_results"] = res
    return _gather(res.results)


# revision 35
# speedup vs baseline: 1.0003x; 1.0003x over previous
"""AdaptiveFlowRouter (soft-MoE routing) on 8 Trainium2 NeuronCores.

Reference computation (per token t of B*S=8192, D=1024, P=8 experts):
    w      = softmax(x @ sel_w.T + sel_b)            # (t, P)
    inten  = sigmoid(x @ int_w.T + int_b)            # (t, 1)
    out    = inten * sum_p w_p * (x @ A_p.T)         # (t, D)
    pattern_entropy     = mean_t(-sum_p w log(w+1e-8))
    flow_intensity_mean = mean(inten)

Sharding: data-parallel over tokens (1024 tokens/core), expert weights
replicated -> no cross-core communication. Host pre-transposes x and
flow_patterns into the [K, *] layouts the TensorEngine wants and
pre-casts the matmul operands to bf16 (fp32 PSUM accumulation).
Each core returns its (1024, 1024) output slice plus a tiny stats tile
(per-token-entropy partials and intensity values); the host finishes
the two scalar means exactly.
"""

from contextlib import ExitStack

import ml_dtypes
import numpy as np

import concourse.bacc as bacc
import concourse.mybir as mybir
import concourse.tile as tile
from concourse.bass_utils import run_bass_kernel_spmd

B, S, D, P = 4, 2048, 1024, 8
N_CORES = 8
TOK = B * S            # 8192 tokens total
T = TOK // N_CORES     # 1024 tokens per core
NT = T // 128          # 8 token tiles (M blocks)
NJ = D // 128          # 8 contraction tiles (K blocks)
IB = 512               # PSUM free-dim block (one bank of f32)
NI = D // IB           # 2 output blocks

BF16 = mybir.dt.bfloat16
F32 = mybir.dt.float32
AF = mybir.ActivationFunctionType
ALU = mybir.AluOpType
AX = mybir.AxisListType

_CACHE = {}

def _build_nc():
    nc = bacc.Bacc()
    xt_d = nc.declare_dram_parameter("xt", [128, NJ * T], BF16, isOutput=False)
    at_d = nc.declare_dram_parameter("at", [P, 128, NJ * D], BF16, isOutput=False)
    si_d = nc.declare_dram_parameter("selint", [128, NJ * 9], BF16, isOutput=False)
    bb_d = nc.declare_dram_parameter("biasb", [128, 9], F32, isOutput=False)
    out_d = nc.declare_dram_parameter("out", [T, D], BF16, isOutput=True)
    st_d = nc.declare_dram_parameter("stats", [128, 16], F32, isOutput=True)

    ctx = ExitStack()
    with ctx:
        # --- Pre-TileContext DMA: expert-0's weights stream during the ~7us
        # framework preamble, issued from the ACT queue (only 4 issues, so
        # ACT still joins the init barrier on time; more pre-issues would
        # delay the barrier and cancel the gain). The consumer carries a
        # manually attached sem wait (added post-scheduling).
        at0_raw = ctx.enter_context(nc.sbuf_tensor("at0_sb", [128, NJ * D], BF16))
        sem_pre = nc.alloc_semaphore("sem_pre")
        HD = D // 2
        AQ = NJ * D // 4
        for q in range(4):
            nc.scalar.dma_start(
                at0_raw[:, q * AQ : (q + 1) * AQ], at_d[0][:, q * AQ : (q + 1) * AQ]
            ).then_inc(sem_pre, 16)

        first_mm_cell = []
        tc = ctx.enter_context(tile.TileContext(nc))
        const = ctx.enter_context(tc.tile_pool(name="const", bufs=1))
        work = ctx.enter_context(tc.tile_pool(name="work", bufs=2))
        psel = ctx.enter_context(tc.tile_pool(name="psel", bufs=2, space="PSUM"))
        pexp = ctx.enter_context(tc.tile_pool(name="pexp", bufs=6, space="PSUM"))

        si = const.tile([128, NJ * 9], BF16, tag="si", name="si")
        nc.sync.dma_start(si[:], si_d[:])
        xt_sb = [
            const.tile([128, D], BF16, tag=f"xt{t}", name=f"xt{t}")
            for t in range(NT)
        ]

        def lhs_ap(tt, jt):
            return xt_sb[tt][:, jt * 128 : (jt + 1) * 128]

        at_sb = [at0_raw]
        for p in range(1, P):
            a = const.tile([128, NJ * D], BF16, tag=f"at{p}", name=f"at{p}")
            at_sb.append(a)

        QD = D // 4
        for t in range(2):
            for h in range(4):
                nc.sync.dma_start(
                    xt_sb[t][:, h * QD : (h + 1) * QD],
                    xt_d[:, t * D + h * QD : t * D + (h + 1) * QD],
                )
        bb = const.tile([128, 9], F32, tag="bb", name="bb")
        nc.sync.dma_start(bb[:], bb_d[:])
        for t in range(2, NT):
            nc.sync.dma_start(xt_sb[t][:], xt_d[:, t * D : (t + 1) * D])
        AH = NJ * D // 2
        for p in range(1, P):
            for h in range(2):
                nc.sync.dma_start(
                    at_sb[p][:, h * AH : (h + 1) * AH],
                    at_d[p][:, h * AH : (h + 1) * AH],
                )
        # stats: cols 0..7 = sum_p w*log(w+1e-8) per token, cols 8..15 = intensity
        stats = const.tile([128, 16], F32, tag="stats", name="stats")
        eps = const.tile([128, 1], F32, tag="eps", name="eps")
        nc.vector.memset(eps[:], 1e-8)
        # per-token mixing weights, pre-scaled by intensity: wsc[t, p] = w_p * inten
        wsc = const.tile([128, NT * P], F32, tag="wsc", name="wsc")
        mix_all = const.tile([128, NT * D], F32, tag="mix", name="mix")
        mix = [mix_all[:, t * D : (t + 1) * D] for t in range(NT)]

        def sel_tt(tt):
            ps = psel.tile([128, 9], F32, tag="ps", name="ps")
            for jt in range(NJ):
                mm = nc.tensor.matmul(
                    ps[:],
                    lhs_ap(tt, jt),
                    si[:, jt * 9 : (jt + 1) * 9],
                    start=(jt == 0),
                    stop=(jt == NJ - 1),
                )


            sc = work.tile([128, 48], F32, tag="softsc", name="softsc")
            logits = sc[:, 0:9]
            negm = sc[:, 9:10]
            e = sc[:, 10:18]
            esum = sc[:, 18:19]
            rcp = sc[:, 19:20]
            coef = sc[:, 20:21]
            wtrue = sc[:, 21:29]
            lw = sc[:, 29:37]
            went = sc[:, 37:45]
            nc.vector.tensor_add(logits, ps[:], bb[:])
            nc.vector.tensor_reduce(
                negm, logits[:, 0:8], axis=AX.X, op=ALU.max, negate=True
            )
            nc.scalar.activation(
                e, logits[:, 0:8], AF.Exp, bias=negm, accum_out=esum
            )
            nc.vector.reciprocal(rcp, esum)
            nc.scalar.activation(
                stats[:, 8 + tt : 9 + tt], logits[:, 8:9], AF.Sigmoid
            )
            nc.vector.tensor_mul(coef, rcp, stats[:, 8 + tt : 9 + tt])
            nc.vector.tensor_scalar_mul(
                wsc[:, tt * P : (tt + 1) * P], e, coef
            )
            nc.vector.tensor_scalar_mul(wtrue, e, rcp)
            nc.scalar.activation(lw, wtrue, AF.Ln, bias=eps[:])
            nc.vector.scalar_tensor_tensor(
                went,
                wtrue,
                1.0,
                lw,
                op0=ALU.bypass,
                op1=ALU.mult,
                accum_out=stats[:, tt : tt + 1],
            )

        def expert_tt(p, tt):
            pt = [pexp.tile([128, IB], F32, tag="pe", name="pe") for _ in range(NI)]
            for jt in range(NJ):
                for ib in range(NI):
                    mm = nc.tensor.matmul(
                        pt[ib][:],
                        lhs_ap(tt, jt),
                        at_sb[p][:, jt * D + ib * IB : jt * D + (ib + 1) * IB],
                        start=(jt == 0),
                        stop=(jt == NJ - 1),
                    )
                    if p == 0 and tt == 0 and jt == 0 and ib == 0:
                        first_mm_cell.append(mm)
            wcol = wsc[:, tt * P + p : tt * P + p + 1]
            if p == P - 1:
                # last expert: fused mul-add emits the final value in bf16
                # directly (DVE converts on write), halving store traffic
                ob = work.tile([128, D], BF16, tag="ob", name="ob", bufs=2)
                for ib in range(NI):
                    nc.vector.scalar_tensor_tensor(
                        ob[:, ib * IB : (ib + 1) * IB],
                        pt[ib][:],
                        wcol,
                        mix[tt][:, ib * IB : (ib + 1) * IB],
                        op0=ALU.mult,
                        op1=ALU.add,
                    )
                    nc.sync.dma_start(
                        out_d[tt * 128 : (tt + 1) * 128, ib * IB : (ib + 1) * IB],
                        ob[:, ib * IB : (ib + 1) * IB],
                    )
            else:
                for ib in range(NI):
                    dst = mix[tt][:, ib * IB : (ib + 1) * IB]
                    if p == 0:
                        nc.vector.tensor_scalar_mul(dst, pt[ib][:], wcol)
                    else:
                        nc.vector.scalar_tensor_tensor(
                            dst, pt[ib][:], wcol, dst, op0=ALU.mult, op1=ALU.add
                        )

        # PE p-state warmup: the first ~14us are input-transfer-bound, and an
        # idle PE drops to 0.65-1.2GHz. Dummy matmuls on a zeroed tile keep
        # the clock ramping so real matmuls start at full speed.
        warm = const.tile([128, 128], BF16, tag="warm", name="warm")
        nc.vector.memset(warm[:], 0.0)

        def warmup(n):
            wp = pexp.tile([128, IB], F32, tag="pe", name="pe")
            for k in range(n):
                nc.tensor.matmul(
                    wp[:, 0:128], warm[:], warm[:], start=True, stop=True
                )

        # Interleave selector and expert-0 per xt chunk so the PE rides the
        # incoming x/weight stream, then run experts 1..7 expert-major.
        warmup(24)
        sel_tt(0)
        sel_tt(1)
        warmup(26)
        expert_tt(0, 0)
        expert_tt(0, 1)
        for c in range(1, 4):
            sel_tt(2 * c)
            sel_tt(2 * c + 1)
            expert_tt(0, 2 * c)
            expert_tt(0, 2 * c + 1)
        nc.sync.dma_start(st_d[:], stats[:])
        for p in range(1, P):
            for tt in range(NT):
                expert_tt(p, tt)
        ctx.close()
        # Attach the external gate after Tile scheduling (the scheduler's
        # deadlock model cannot see pre-region DMA increments).
        first_mm_cell[0]._wait_ge(sem_pre, 64)
        nc.clear_and_free_semaphores([sem_pre])
    nc.finalize()
    return nc


def _prep_in_maps(x, flow_patterns, sel_w, sel_b, int_w, int_b):
    bf = ml_dtypes.bfloat16
    f32 = np.float32
    x = np.asarray(x, f32)
    flow_patterns = np.asarray(flow_patterns, f32)
    sel_w = np.asarray(sel_w, f32)
    sel_b = np.asarray(sel_b, f32)
    int_w = np.asarray(int_w, f32)
    int_b = np.asarray(int_b, f32)

    # A_p^T in [j, i] layout as the per-expert SBUF image:
    # at_h[p, part, jt*D + i] = A_p[i, jt*128 + part]
    at_h = np.ascontiguousarray(
        flow_patterns.transpose(0, 2, 1)
        .reshape(P, NJ, 128, D)
        .transpose(0, 2, 1, 3)
        .reshape(P, 128, NJ * D)
    ).astype(bf)
    si_h = np.ascontiguousarray(
        np.concatenate([sel_w, int_w], axis=0)
        .T.reshape(NJ, 128, 9)
        .transpose(1, 0, 2)
        .reshape(128, NJ * 9)
    ).astype(bf)
    bb_h = np.ascontiguousarray(
        np.broadcast_to(np.concatenate([sel_b, int_b], axis=0)[None, :], (128, 9))
    ).astype(f32)

    xf = x.reshape(TOK, D)
    in_maps = []
    for c in range(N_CORES):
        xt_h = np.ascontiguousarray(
            xf[c * T : (c + 1) * T]
            .T.reshape(NJ, 128, NT, 128)
            .transpose(1, 2, 0, 3)
            .reshape(128, NJ * T)
        ).astype(bf)
        in_maps.append({"xt": xt_h, "at": at_h, "selint": si_h, "biasb": bb_h})
    return in_maps


def _gather(results):
    out = (
        np.concatenate([r["out"] for r in results], axis=0)
        .astype(np.float32)
        .reshape(B, S, D)
    )
    ent_sum = sum(float(r["stats"][:, 0:8].sum(dtype=np.float64)) for r in results)
    int_sum = sum(float(r["stats"][:, 8:16].sum(dtype=np.float64)) for r in results)
    pattern_entropy = np.float32(-ent_sum / TOK)
    flow_intensity_mean = np.float32(int_sum / TOK)
    return out, pattern_entropy, flow_intensity_mean


def kernel(x, flow_patterns, sel_w, sel_b, int_w, int_b):
    if "nc" not in _CACHE:
        _CACHE["nc"] = _build_nc()
    nc = _CACHE["nc"]
    in_maps = _prep_in_maps(x, flow_patterns, sel_w, sel_b, int_w, int_b)
    res = run_bass_kernel_spmd(nc, in_maps, core_ids=list(range(N_CORES)))
    _CACHE["last_results"] = res
    return _gather(res.results)


# revision 36
# speedup vs baseline: 1.0173x; 1.0170x over previous
"""AdaptiveFlowRouter (soft-MoE routing) on 8 Trainium2 NeuronCores.

Reference computation (per token t of B*S=8192, D=1024, P=8 experts):
    w      = softmax(x @ sel_w.T + sel_b)            # (t, P)
    inten  = sigmoid(x @ int_w.T + int_b)            # (t, 1)
    out    = inten * sum_p w_p * (x @ A_p.T)         # (t, D)
    pattern_entropy     = mean_t(-sum_p w log(w+1e-8))
    flow_intensity_mean = mean(inten)

Sharding: data-parallel over tokens (1024 tokens/core), expert weights
replicated -> no cross-core communication. Host pre-transposes x and
flow_patterns into the [K, *] layouts the TensorEngine wants and
pre-casts the matmul operands to bf16 (fp32 PSUM accumulation).
Each core returns its (1024, 1024) output slice plus a tiny stats tile
(per-token-entropy partials and intensity values); the host finishes
the two scalar means exactly.
"""

from contextlib import ExitStack

import ml_dtypes
import numpy as np

import concourse.bacc as bacc
import concourse.mybir as mybir
import concourse.tile as tile
from concourse.bass_utils import run_bass_kernel_spmd

B, S, D, P = 4, 2048, 1024, 8
N_CORES = 8
TOK = B * S            # 8192 tokens total
T = TOK // N_CORES     # 1024 tokens per core
NT = T // 128          # 8 token tiles (M blocks)
NJ = D // 128          # 8 contraction tiles (K blocks)
IB = 512               # PSUM free-dim block (one bank of f32)
NI = D // IB           # 2 output blocks

BF16 = mybir.dt.bfloat16
F32 = mybir.dt.float32
AF = mybir.ActivationFunctionType
ALU = mybir.AluOpType
AX = mybir.AxisListType

_CACHE = {}

def _build_nc():
    nc = bacc.Bacc()
    xt_d = nc.declare_dram_parameter("xt", [128, NJ * T], BF16, isOutput=False)
    at_d = nc.declare_dram_parameter("at", [P, 128, NJ * D], BF16, isOutput=False)
    si_d = nc.declare_dram_parameter("selint", [128, NJ * 9], BF16, isOutput=False)
    bb_d = nc.declare_dram_parameter("biasb", [128, 9], F32, isOutput=False)
    out_d = nc.declare_dram_parameter("out", [T, D], BF16, isOutput=True)
    st_d = nc.declare_dram_parameter("stats", [128, 16], F32, isOutput=True)

    ctx = ExitStack()
    with ctx:
        # --- Pre-TileContext DMA: expert-0's weights stream during the ~7us
        # framework preamble, issued from the ACT queue (only 4 issues, so
        # ACT still joins the init barrier on time; more pre-issues would
        # delay the barrier and cancel the gain). The consumer carries a
        # manually attached sem wait (added post-scheduling).
        at0_raw = ctx.enter_context(nc.sbuf_tensor("at0_sb", [128, NJ * D], BF16))
        sem_pre = nc.alloc_semaphore("sem_pre")
        HD = D // 2
        AQ = NJ * D // 4
        for q in range(4):
            nc.scalar.dma_start(
                at0_raw[:, q * AQ : (q + 1) * AQ], at_d[0][:, q * AQ : (q + 1) * AQ]
            ).then_inc(sem_pre, 16)

        first_mm_cell = []
        tc = ctx.enter_context(tile.TileContext(nc))
        const = ctx.enter_context(tc.tile_pool(name="const", bufs=1))
        work = ctx.enter_context(tc.tile_pool(name="work", bufs=2))
        psel = ctx.enter_context(tc.tile_pool(name="psel", bufs=2, space="PSUM"))
        pexp = ctx.enter_context(tc.tile_pool(name="pexp", bufs=6, space="PSUM"))

        si = const.tile([128, NJ * 9], BF16, tag="si", name="si")
        nc.sync.dma_start(si[:], si_d[:])
        xt_sb = [
            const.tile([128, D], BF16, tag=f"xt{t}", name=f"xt{t}")
            for t in range(NT)
        ]

        def lhs_ap(tt, jt):
            return xt_sb[tt][:, jt * 128 : (jt + 1) * 128]

        at_sb = [at0_raw]
        for p in range(1, P):
            a = const.tile([128, NJ * D], BF16, tag=f"at{p}", name=f"at{p}")
            at_sb.append(a)

        for t in range(2):
            for h in range(2):
                nc.sync.dma_start(
                    xt_sb[t][:, h * HD : (h + 1) * HD],
                    xt_d[:, t * D + h * HD : t * D + (h + 1) * HD],
                )
        bb = const.tile([128, 9], F32, tag="bb", name="bb")
        nc.sync.dma_start(bb[:], bb_d[:])
        for t in range(2, NT):
            nc.sync.dma_start(xt_sb[t][:], xt_d[:, t * D : (t + 1) * D])
        AH = NJ * D // 2
        for p in range(1, P):
            for h in range(2):
                nc.sync.dma_start(
                    at_sb[p][:, h * AH : (h + 1) * AH],
                    at_d[p][:, h * AH : (h + 1) * AH],
                )
        # stats: cols 0..7 = sum_p w*log(w+1e-8) per token, cols 8..15 = intensity
        stats = const.tile([128, 16], F32, tag="stats", name="stats")
        eps = const.tile([128, 1], F32, tag="eps", name="eps")
        nc.vector.memset(eps[:], 1e-8)
        # per-token mixing weights, pre-scaled by intensity: wsc[t, p] = w_p * inten
        wsc = const.tile([128, NT * P], F32, tag="wsc", name="wsc")
        mix_all = const.tile([128, NT * D], F32, tag="mix", name="mix")
        mix = [mix_all[:, t * D : (t + 1) * D] for t in range(NT)]

        def sel_tt(tt):
            ps = psel.tile([128, 9], F32, tag="ps", name="ps")
            for jt in range(NJ):
                mm = nc.tensor.matmul(
                    ps[:],
                    lhs_ap(tt, jt),
                    si[:, jt * 9 : (jt + 1) * 9],
                    start=(jt == 0),
                    stop=(jt == NJ - 1),
                )


            sc = work.tile([128, 48], F32, tag="softsc", name="softsc")
            logits = sc[:, 0:9]
            negm = sc[:, 9:10]
            e = sc[:, 10:18]
            esum = sc[:, 18:19]
            rcp = sc[:, 19:20]
            coef = sc[:, 20:21]
            wtrue = sc[:, 21:29]
            lw = sc[:, 29:37]
            went = sc[:, 37:45]
            nc.vector.tensor_add(logits, ps[:], bb[:])
            nc.vector.tensor_reduce(
                negm, logits[:, 0:8], axis=AX.X, op=ALU.max, negate=True
            )
            nc.scalar.activation(
                e, logits[:, 0:8], AF.Exp, bias=negm, accum_out=esum
            )
            nc.vector.reciprocal(rcp, esum)
            nc.scalar.activation(
                stats[:, 8 + tt : 9 + tt], logits[:, 8:9], AF.Sigmoid
            )
            nc.vector.tensor_mul(coef, rcp, stats[:, 8 + tt : 9 + tt])
            nc.vector.tensor_scalar_mul(
                wsc[:, tt * P : (tt + 1) * P], e, coef
            )
            nc.vector.tensor_scalar_mul(wtrue, e, rcp)
            nc.scalar.activation(lw, wtrue, AF.Ln, bias=eps[:])
            nc.vector.scalar_tensor_tensor(
                went,
                wtrue,
                1.0,
                lw,
                op0=ALU.bypass,
                op1=ALU.mult,
                accum_out=stats[:, tt : tt + 1],
            )

        def expert_tt(p, tt):
            pt = [pexp.tile([128, IB], F32, tag="pe", name="pe") for _ in range(NI)]
            for jt in range(NJ):
                for ib in range(NI):
                    mm = nc.tensor.matmul(
                        pt[ib][:],
                        lhs_ap(tt, jt),
                        at_sb[p][:, jt * D + ib * IB : jt * D + (ib + 1) * IB],
                        start=(jt == 0),
                        stop=(jt == NJ - 1),
                    )
                    if p == 0 and tt == 0 and jt == 0 and ib == 0:
                        first_mm_cell.append(mm)
            wcol = wsc[:, tt * P + p : tt * P + p + 1]
            if p == P - 1:
                # last expert: fused mul-add emits the final value in bf16
                # directly (DVE converts on write), halving store traffic
                ob = work.tile([128, D], BF16, tag="ob", name="ob", bufs=2)
                for ib in range(NI):
                    nc.vector.scalar_tensor_tensor(
                        ob[:, ib * IB : (ib + 1) * IB],
                        pt[ib][:],
                        wcol,
                        mix[tt][:, ib * IB : (ib + 1) * IB],
                        op0=ALU.mult,
                        op1=ALU.add,
                    )
                    nc.sync.dma_start(
                        out_d[tt * 128 : (tt + 1) * 128, ib * IB : (ib + 1) * IB],
                        ob[:, ib * IB : (ib + 1) * IB],
                    )
            else:
                for ib in range(NI):
                    dst = mix[tt][:, ib * IB : (ib + 1) * IB]
                    if p == 0:
                        nc.vector.tensor_scalar_mul(dst, pt[ib][:], wcol)
                    else:
                        nc.vector.scalar_tensor_tensor(
                            dst, pt[ib][:], wcol, dst, op0=ALU.mult, op1=ALU.add
                        )

        # PE p-state warmup: the first ~14us are input-transfer-bound, and an
        # idle PE drops to 0.65-1.2GHz. Dummy matmuls on a zeroed tile keep
        # the clock ramping so real matmuls start at full speed.
        warm = const.tile([128, 128], BF16, tag="warm", name="warm")
        nc.vector.memset(warm[:], 0.0)

        def warmup(n):
            wp = pexp.tile([128, IB], F32, tag="pe", name="pe")
            for k in range(n):
                nc.tensor.matmul(
                    wp[:, 0:128], warm[:], warm[:], start=True, stop=True
                )

        # Interleave selector and expert-0 per xt chunk so the PE rides the
        # incoming x/weight stream, then run experts 1..7 expert-major.
        warmup(24)
        sel_tt(0)
        sel_tt(1)
        warmup(26)
        expert_tt(0, 0)
        expert_tt(0, 1)
        for c in range(1, 4):
            sel_tt(2 * c)
            sel_tt(2 * c + 1)
            expert_tt(0, 2 * c)
            expert_tt(0, 2 * c + 1)
        nc.sync.dma_start(st_d[:], stats[:])
        for p in range(1, P):
            for tt in range(NT):
                expert_tt(p, tt)
        ctx.close()
        # Attach the external gate after Tile scheduling (the scheduler's
        # deadlock model cannot see pre-region DMA increments).
        first_mm_cell[0]._wait_ge(sem_pre, 64)
        nc.clear_and_free_semaphores([sem_pre])
    nc.finalize()
    return nc


def _prep_in_maps(x, flow_patterns, sel_w, sel_b, int_w, int_b):
    bf = ml_dtypes.bfloat16
    f32 = np.float32
    x = np.asarray(x, f32)
    flow_patterns = np.asarray(flow_patterns, f32)
    sel_w = np.asarray(sel_w, f32)
    sel_b = np.asarray(sel_b, f32)
    int_w = np.asarray(int_w, f32)
    int_b = np.asarray(int_b, f32)

    # A_p^T in [j, i] layout as the per-expert SBUF image:
    # at_h[p, part, jt*D + i] = A_p[i, jt*128 + part]
    at_h = np.ascontiguousarray(
        flow_patterns.transpose(0, 2, 1)
        .reshape(P, NJ, 128, D)
        .transpose(0, 2, 1, 3)
        .reshape(P, 128, NJ * D)
    ).astype(bf)
    si_h = np.ascontiguousarray(
        np.concatenate([sel_w, int_w], axis=0)
        .T.reshape(NJ, 128, 9)
        .transpose(1, 0, 2)
        .reshape(128, NJ * 9)
    ).astype(bf)
    bb_h = np.ascontiguousarray(
        np.broadcast_to(np.concatenate([sel_b, int_b], axis=0)[None, :], (128, 9))
    ).astype(f32)

    xf = x.reshape(TOK, D)
    in_maps = []
    for c in range(N_CORES):
        xt_h = np.ascontiguousarray(
            xf[c * T : (c + 1) * T]
            .T.reshape(NJ, 128, NT, 128)
            .transpose(1, 2, 0, 3)
            .reshape(128, NJ * T)
        ).astype(bf)
        in_maps.append({"xt": xt_h, "at": at_h, "selint": si_h, "biasb": bb_h})
    return in_maps


def _gather(results):
    out = (
        np.concatenate([r["out"] for r in results], axis=0)
        .astype(np.float32)
        .reshape(B, S, D)
    )
    ent_sum = sum(float(r["stats"][:, 0:8].sum(dtype=np.float64)) for r in results)
    int_sum = sum(float(r["stats"][:, 8:16].sum(dtype=np.float64)) for r in results)
    pattern_entropy = np.float32(-ent_sum / TOK)
    flow_intensity_mean = np.float32(int_sum / TOK)
    return out, pattern_entropy, flow_intensity_mean


def kernel(x, flow_patterns, sel_w, sel_b, int_w, int_b):
    if "nc" not in _CACHE:
        _CACHE["nc"] = _build_nc()
    nc = _CACHE["nc"]
    in_maps = _prep_in_maps(x, flow_patterns, sel_w, sel_b, int_w, int_b)
    res = run_bass_kernel_spmd(nc, in_maps, core_ids=list(range(N_CORES)))
    _CACHE["last_results"] = res
    return _gather(res.results)
